# revision 14
# baseline (speedup 1.0000x reference)
"""Trainium2 Bass kernel for nn_DAttention:
out[b,c,d,h,w] = x[b,c,d,h,w] * mean_{c,h,w}(x[b,:,d,:,:]).

Sharding: pure data parallel over batch B=8 -> one batch per NeuronCore.
Numerics: HBM I/O in bf16 (the host converts f32->bf16 in, bf16->f32
out); the mean is accumulated in fp32 (ACT accum_out + fp32 matmuls),
so the element error is ~2 bf16 roundings (~0.4%) -- far inside the
2e-2 gate. This halves HBM traffic vs f32: 32 MiB in + 32 MiB out per
core, the memory-roofline minimum for this regime.

Per core, loop over 16 PAIRS of d-slices (2 MiB per pair in bf16).
SBUF tile [128, 8192] bf16: partition p = c*4 + hg, and the host
interleaves the pair so row p = slice 2k's 4096 elems ++ slice 2k+1's
4096 elems, one contiguous 16 KiB DRAM run per row. 16 KiB power-of-2
descriptors run at DMA line rate +~1% (8 KiB pay ~3%, odd sizes like
7232 B pay 30%, and row counts that don't split evenly across the 16
SDMA engines fall off the descriptor balancer entirely -- all
measured), and pairing halves the dispatch + semaphore overhead on the
issuing engines.

Engine schedule per pair (balanced so the post-load tail stays close
to store-DMA-paced; each sub-slice reduced separately):
  ACT: 2x activation-Copy of xt[:, base:base+A] into a dead PSUM
       scratch with accum_out -> fp32 column sums (only Matmul/Memset
       may write bf16 to PSUM, so the scratch is f32)
  DVE: 2x tensor_reduce(add) of xt[:, base+A:base+4096] -> fp32 sums
  PE : 2x two accumulated fp32 matmuls vs a constant 128x128 matrix
       of 1/2^19 -> each slice's mean broadcast to a PSUM [128,1]
  ACT: 2x tiny copy mean PSUM->SBUF
  DVE: 2x tensor_scalar multiply (bf16) into the pair out tile
  DMA: one 2 MiB load (SP ring), one 2 MiB store (ACT ring) per pair
"""
import numpy as np
import ml_dtypes

import concourse.bacc as bacc
import concourse.tile as tile
import concourse.mybir as mybir
from concourse.bass_utils import run_bass_kernel_spmd

BF16 = ml_dtypes.bfloat16

B, C, D, H, W = 8, 32, 32, 128, 128
DP = D // 2             # 16 slice pairs
HG, HL = 4, 32          # H split: partition dim = C*HG = 128
P = C * HG              # 128 partitions
F = HL * W              # 4096 free elements per partition per slice
FP = 2 * F              # 8192 per pair (16 KiB rows)
N_RED = C * H * W       # 524288 = 2**19 elements reduced per (b, d)
RECIP = 1.0 / N_RED     # exact in fp32
A_SPLIT = 2816          # ACT reduces [base:base+A], DVE the rest

_NC = None


def _build_nc(xin_bufs=6, out_bufs=2):
    nc = bacc.Bacc("TRN2", target_bir_lowering=False, debug=False)
    xp5 = nc.dram_tensor("x", [DP, P, FP], mybir.dt.bfloat16, kind="ExternalInput")
    op5 = nc.dram_tensor("out", [DP, P, FP], mybir.dt.bfloat16, kind="ExternalOutput")
    with tile.TileContext(nc) as tc:
        with (
            tc.tile_pool(name="xin", bufs=xin_bufs) as xpool,
            tc.tile_pool(name="oout", bufs=out_bufs) as opool,
            tc.tile_pool(name="small", bufs=8) as spool,
            tc.tile_pool(name="psum", bufs=2, space="PSUM") as ppool,
            tc.tile_pool(name="psc", bufs=1, space="PSUM") as scpool,
            tc.tile_pool(name="const", bufs=1) as cpool,
        ):
            recip = cpool.tile([P, P], mybir.dt.float32)
            nc.gpsimd.memset(recip[:], RECIP)
            for k in range(DP):
                xt = xpool.tile([P, FP], mybir.dt.bfloat16, tag="xt")
                nc.sync.dma_start(xt[:], xp5[k])
                ot = opool.tile([P, FP], mybir.dt.bfloat16, tag="ot")
                for s in range(2):
                    base = s * F
                    csa = spool.tile([P, 1], mybir.dt.float32, tag=f"csa{s}")
                    csb = spool.tile([P, 1], mybir.dt.float32, tag=f"csb{s}")
                    scratch = scpool.tile([P, A_SPLIT], mybir.dt.float32, tag="sc")
                    nc.scalar.activation(
                        scratch[:], xt[:, base : base + A_SPLIT],
                        mybir.ActivationFunctionType.Copy, accum_out=csa[:],
                    )
                    nc.vector.tensor_reduce(
                        csb[:], xt[:, base + A_SPLIT : base + F],
                        mybir.AxisListType.X, mybir.AluOpType.add,
                    )
                    dv = ppool.tile([P, 1], mybir.dt.float32, tag="dv")
                    nc.tensor.matmul(dv[:], recip[:], csa[:], start=True, stop=False)
                    nc.tensor.matmul(dv[:], recip[:], csb[:], start=False, stop=True)
                    dvs = spool.tile([P, 1], mybir.dt.float32, tag=f"dvs{s}")
                    nc.scalar.copy(dvs[:], dv[:])
                    nc.vector.tensor_scalar_mul(
                        ot[:, base : base + F], xt[:, base : base + F], dvs[:]
                    )
                nc.scalar.dma_start(op5[k], ot[:])
    nc.compile()
    return nc


def _get_nc():
    global _NC
    if _NC is None:
        _NC = _build_nc()
    return _NC


def _deal_in(x_core: np.ndarray):
    """[C,D,H,W] f32 -> xp [DP, 128, 8192] bf16 (pair-interleaved rows)."""
    z = (
        x_core.astype(BF16)
        .reshape(C, D, HG, HL * W)
        .transpose(1, 0, 2, 3)
        .reshape(D, P, F)
    )
    xp = z.reshape(DP, 2, P, F).transpose(0, 2, 1, 3).reshape(DP, P, FP)
    return np.ascontiguousarray(xp)


def _deal_out(op_core: np.ndarray):
    """op [DP, 128, 8192] bf16 -> [C,D,H,W] f32."""
    rows = op_core.reshape(DP, P, 2, F).transpose(0, 2, 1, 3).reshape(D, C, HG, F)
    return rows.transpose(1, 0, 2, 3).reshape(C, D, H, W).astype(np.float32)


def run(x: np.ndarray, trace: bool = False, tmpdir: str | None = None):
    """Run on 8 NeuronCores; returns (out, BassKernelResults)."""
    x = np.asarray(x)
    assert x.shape == (B, C, D, H, W), x.shape
    nc = _get_nc()
    in_maps = [{"x": _deal_in(x[b])} for b in range(B)]
    res = run_bass_kernel_spmd(
        nc, in_maps, core_ids=list(range(B)), trace=trace, tmpdir=tmpdir
    )
    out = np.stack([_deal_out(r["out"]) for r in res.results])
    return out, res


def kernel(x: np.ndarray) -> np.ndarray:
    out, _ = run(x)
    return out


# revision 15
# speedup vs baseline: 1.1731x; 1.1731x over previous
"""Trainium2 Bass kernel for nn_DAttention:
out[b,c,d,h,w] = x[b,c,d,h,w] * mean_{c,h,w}(x[b,:,d,:,:]).

Sharding: pure data parallel over batch B=8 -> one batch per NeuronCore.
Numerics: HBM I/O in bf16 (the host converts f32->bf16 in, bf16->f32
out); the mean is accumulated in fp32 (ACT accum_out + fp32 matmuls),
so the element error is ~2 bf16 roundings (~0.4%) -- far inside the
2e-2 gate. This halves HBM traffic vs f32: 32 MiB in + 32 MiB out per
core, the memory-roofline minimum for this regime.

Per core, loop over 16 PAIRS of d-slices (2 MiB per pair in bf16).
SBUF tile [128, 8192] bf16: partition p = c*4 + hg, and the host
interleaves the pair so row p = slice 2k's 4096 elems ++ slice 2k+1's
4096 elems, one contiguous 16 KiB DRAM run per row. 16 KiB power-of-2
descriptors run at DMA line rate +~1% (8 KiB pay ~3%, odd sizes like
7232 B pay 30%, and row counts that don't split evenly across the 16
SDMA engines fall off the descriptor balancer entirely -- all
measured), and pairing halves the dispatch + semaphore overhead on the
issuing engines.

Engine schedule per pair (balanced so the post-load tail stays close
to store-DMA-paced; each sub-slice reduced separately):
  ACT: 2x activation-Copy of xt[:, base:base+A] into a dead PSUM
       scratch with accum_out -> fp32 column sums (only Matmul/Memset
       may write bf16 to PSUM, so the scratch is f32)
  DVE: 2x tensor_reduce(add) of xt[:, base+A:base+4096] -> fp32 sums
  PE : 2x two accumulated fp32 matmuls vs a constant 128x128 matrix
       of 1/2^19 -> each slice's mean broadcast to a PSUM [128,1]
  ACT: 2x tiny copy mean PSUM->SBUF
  DVE: 2x tensor_scalar multiply (bf16) into the pair out tile
  DMA: one 2 MiB load (SP ring), one 2 MiB store (ACT ring) per pair
"""
import numpy as np
import ml_dtypes

import concourse.bacc as bacc
import concourse.tile as tile
import concourse.mybir as mybir
from concourse.bass_utils import run_bass_kernel_spmd

BF16 = ml_dtypes.bfloat16

B, C, D, H, W = 8, 32, 32, 128, 128
DP = D // 2             # 16 slice pairs
HG, HL = 4, 32          # H split: partition dim = C*HG = 128
P = C * HG              # 128 partitions
F = HL * W              # 4096 free elements per partition per slice
FP = 2 * F              # 8192 per pair (16 KiB rows)
N_RED = C * H * W       # 524288 = 2**19 elements reduced per (b, d)
RECIP = 1.0 / N_RED     # exact in fp32
A_SPLIT = 2816          # ACT reduces [base:base+A], DVE the rest

_NC = None


def _build_nc(xin_bufs=8, out_bufs=3):
    nc = bacc.Bacc("TRN2", target_bir_lowering=False, debug=False)
    xp5 = nc.dram_tensor("x", [DP, P, FP], mybir.dt.bfloat16, kind="ExternalInput")
    op5 = nc.dram_tensor("out", [DP, P, FP], mybir.dt.bfloat16, kind="ExternalOutput")
    with tile.TileContext(nc) as tc:
        with (
            tc.tile_pool(name="xin", bufs=xin_bufs) as xpool,
            tc.tile_pool(name="oout", bufs=out_bufs) as opool,
            tc.tile_pool(name="small", bufs=8) as spool,
            tc.tile_pool(name="psum", bufs=2, space="PSUM") as ppool,
            tc.tile_pool(name="psc", bufs=1, space="PSUM") as scpool,
            tc.tile_pool(name="const", bufs=1) as cpool,
        ):
            recip = cpool.tile([P, P], mybir.dt.float32)
            nc.gpsimd.memset(recip[:], RECIP)
            for k in range(DP):
                xt = xpool.tile([P, FP], mybir.dt.bfloat16, tag="xt")
                nc.sync.dma_start(xt[:], xp5[k])
                ot = opool.tile([P, FP], mybir.dt.bfloat16, tag="ot")
                for s in range(2):
                    base = s * F
                    csa = spool.tile([P, 1], mybir.dt.float32, tag=f"csa{s}")
                    csb = spool.tile([P, 1], mybir.dt.float32, tag=f"csb{s}")
                    scratch = scpool.tile([P, A_SPLIT], mybir.dt.float32, tag="sc")
                    nc.scalar.activation(
                        scratch[:], xt[:, base : base + A_SPLIT],
                        mybir.ActivationFunctionType.Copy, accum_out=csa[:],
                    )
                    nc.vector.tensor_reduce(
                        csb[:], xt[:, base + A_SPLIT : base + F],
                        mybir.AxisListType.X, mybir.AluOpType.add,
                    )
                    dv = ppool.tile([P, 1], mybir.dt.float32, tag="dv")
                    nc.tensor.matmul(dv[:], recip[:], csa[:], start=True, stop=False)
                    nc.tensor.matmul(dv[:], recip[:], csb[:], start=False, stop=True)
                    dvs = spool.tile([P, 1], mybir.dt.float32, tag=f"dvs{s}")
                    nc.scalar.copy(dvs[:], dv[:])
                    nc.vector.tensor_scalar_mul(
                        ot[:, base : base + F], xt[:, base : base + F], dvs[:]
                    )
                nc.scalar.dma_start(op5[k], ot[:])
    nc.compile()
    return nc


def _get_nc():
    global _NC
    if _NC is None:
        _NC = _build_nc()
    return _NC


def _deal_in(x_core: np.ndarray):
    """[C,D,H,W] f32 -> xp [DP, 128, 8192] bf16 (pair-interleaved rows)."""
    z = (
        x_core.astype(BF16)
        .reshape(C, D, HG, HL * W)
        .transpose(1, 0, 2, 3)
        .reshape(D, P, F)
    )
    xp = z.reshape(DP, 2, P, F).transpose(0, 2, 1, 3).reshape(DP, P, FP)
    return np.ascontiguousarray(xp)


def _deal_out(op_core: np.ndarray):
    """op [DP, 128, 8192] bf16 -> [C,D,H,W] f32."""
    rows = op_core.reshape(DP, P, 2, F).transpose(0, 2, 1, 3).reshape(D, C, HG, F)
    return rows.transpose(1, 0, 2, 3).reshape(C, D, H, W).astype(np.float32)


def run(x: np.ndarray, trace: bool = False, tmpdir: str | None = None):
    """Run on 8 NeuronCores; returns (out, BassKernelResults)."""
    x = np.asarray(x)
    assert x.shape == (B, C, D, H, W), x.shape
    nc = _get_nc()
    in_maps = [{"x": _deal_in(x[b])} for b in range(B)]
    res = run_bass_kernel_spmd(
        nc, in_maps, core_ids=list(range(B)), trace=trace, tmpdir=tmpdir
    )
    out = np.stack([_deal_out(r["out"]) for r in res.results])
    return out, res


def kernel(x: np.ndarray) -> np.ndarray:
    out, _ = run(x)
    return out
